# revision 27
# baseline (speedup 1.0000x reference)
"""BitNet MNIST MLP forward on 8 Trainium2 NeuronCores (pure data parallel).

Reference math (per _bitlinear): out = (x/sx) @ w_q.T * sx with per-row
sx = max(|x|) -- the activation scale cancels exactly, so we compute
x @ w_q.T directly.  Ternary w_q is precomputed on host (exact in bf16).

Per-core dataflow (batch shard 8192 rows; work items of 512 batch cols,
with the first and last two chunks split into 256-col halves to shorten
the pipeline fill/drain dependency chains):
  activations live feature-major [feat_part(128) x batch_free] in SBUF, so
  every layer's matmul contracts features on partitions with stationary
  (pre-transposed) weights and NO on-chip transposes.  Ternary weights
  ship as fp8e4 (exact for {-1,0,+1}): halves weight DMA and gets the 4x
  fast-weight-load path.  Weight DMAs ride the ACT HWDGE ring, x the sync
  ring -- the two streams run concurrently through the startup backlog.
  L1 contraction: 6 full 128-row k-tiles + a 16-row leftover (784 = 6*128
  + 16).  The leftover is NOT padded to a full tile: the 16 rows are
  replicated host-side at partition offsets 0/32/64/96 and contracted by
  4 concurrent 32-row matmuls (tile_position row packing), saving 6 of 8
  full-cost matmul slots per chunk.
  RMS mean(h^2) over the 1024 features = ones(1/1024)-matmul accumulated
  over the 8 feature tiles -> replicated [128, BS] PSUM value.  Each
  ones-matmul is emitted AFTER a following matmul block so the PE never
  stalls on the DVE square/reduce tree; the L2 block is emitted in two
  halves around norm-L1 so the L1 ones-matmul fires mid-block and the
  rsqrt chain reaches DVE with slack.  The L2 square-tree is emitted
  after norm-L1's chain (off the h1out critical path).
  rsqrt = int bit-trick seed + 1 Newton iteration on DVE.
  gelu(h*rinv*g): rinv via per-tile DVE tensor-tensor (2x mode, in-place
  into hraw), g via per-partition ACT scale.  Superstep order per item s:
    [L1(s) mm] [norm L2(s-2)] [L2(s-1) mm 1st half] [norm L1(s)]
    [L2(s-1) mm 2nd half] [L2 tree] [L3(s-2)]
  Drain (last two items): fused fine norm2+L3 (each L3 col-strip fires as
  soon as its two gelu tiles land) plus tiny no-dep filler matmuls that
  keep the HAM activity window busy so the PE clock stays at 2.4 GHz.
"""

import os
from contextlib import ExitStack

import numpy as np
import ml_dtypes

import concourse.bacc as bacc
import concourse.bass as bass
import concourse.mybir as mybir
import concourse.tile as tile
from concourse.bass_utils import run_bass_kernel_spmd

N_CORES = 8
B, IN, H, OUT = 65536, 784, 1024, 10
BPC = B // N_CORES  # 8192 rows per core
KF = 6              # full 128-row contraction tiles of x (768 features)
KL = IN - KF * 128  # 16 leftover features
K2 = H // 128       # 8 contraction tiles, layers 2/3
HO = H // 128       # 8 output-feature tiles
BS = 512            # batch columns per full chunk
NB = BPC // BS      # 16 full chunks
EPS_Q = 1e-5
MAGIC = 0x5F3759DF

F32 = mybir.dt.float32
BF16 = mybir.dt.bfloat16
FP8 = mybir.dt.float8e4  # ternary weights are exact in e4m3
I32 = mybir.dt.int32
ALU = mybir.AluOpType
ACTF = mybir.ActivationFunctionType

_cache = {}
LAST_RESULTS = None  # test.py reads exec_time_ns off this


def _build(g_is_one=True):
    # Bacc (not raw Bass): its compile() runs generate_event_semaphores(),
    # which splits multi-wait sync_infos down to the 1-wait HW limit.
    nc = bacc.Bacc("TRN2", target_bir_lowering=False, debug=False, num_devices=N_CORES)

    xt = nc.dram_tensor("xt", [KF * 128, BPC], BF16, kind="ExternalInput").ap()
    xlt = nc.dram_tensor("xlt", [128, BPC], BF16, kind="ExternalInput").ap()
    w1t = nc.dram_tensor("w1t", [KF * 128, H], FP8, kind="ExternalInput").ap()
    w1lt = nc.dram_tensor("w1lt", [128, 2 * 128], FP8, kind="ExternalInput").ap()
    w2t = nc.dram_tensor("w2t", [H, H], FP8, kind="ExternalInput").ap()
    w3t = nc.dram_tensor("w3t", [128, K2 * OUT], FP8, kind="ExternalInput").ap()
    g1 = nc.dram_tensor("g1", [128, HO], F32, kind="ExternalInput").ap()
    g2 = nc.dram_tensor("g2", [128, HO], F32, kind="ExternalInput").ap()
    outt = nc.dram_tensor("outt", [OUT, BPC], F32, kind="ExternalOutput").ap()

    # work items: the first and last two chunks are split into 256-col
    # halves (shorter dependency chains through pipeline fill and drain;
    # fine-mode norm there), the middle 13 chunks are full 512-col items.
    items = [(0, BS // 2), (BS // 2, BS // 2)]
    items += [(c * BS, BS) for c in range(1, NB - 2)]
    for c in (NB - 2, NB - 1):
        items += [(c * BS, BS // 2), (c * BS + BS // 2, BS // 2)]
    NI = len(items)
    # fine/fused mode only for the last two items (the exposed drain)
    FINE = {NI - 2, NI - 1}

    with tile.TileContext(nc) as tc, ExitStack() as ctx:
        wp = ctx.enter_context(tc.tile_pool(name="weights", bufs=1))
        xp = ctx.enter_context(tc.tile_pool(name="x", bufs=3))
        hp = ctx.enter_context(tc.tile_pool(name="h", bufs=3))
        hsp = ctx.enter_context(tc.tile_pool(name="hsq", bufs=1))
        hq1 = ctx.enter_context(tc.tile_pool(name="h1out", bufs=2))
        hq2 = ctx.enter_context(tc.tile_pool(name="h2out", bufs=2))
        rp = ctx.enter_context(tc.tile_pool(name="rsq", bufs=1))
        op = ctx.enter_context(tc.tile_pool(name="out", bufs=3))
        pp = ctx.enter_context(tc.tile_pool(name="ps", bufs=5, space="PSUM"))
        sp = ctx.enter_context(tc.tile_pool(name="ssq", bufs=1, space="PSUM"))
        p3 = ctx.enter_context(tc.tile_pool(name="ps3", bufs=2, space="PSUM"))

        w1sb = wp.tile([128, KF, H], FP8)
        w1lsb = wp.tile([128, 2 * 128], FP8)
        g1sb = wp.tile([128, HO], F32)
        ones = wp.tile([128, 128], BF16)
        w2sb = wp.tile([128, K2, H], FP8)
        w3sb = wp.tile([128, K2, OUT], FP8)
        g2sb = wp.tile([128, HO], F32)

        xt_r = xt.rearrange("(k p) b -> p k b", p=128)
        w2t_r = w2t.rearrange("(k p) h -> p k h", p=128)
        w3t_r = w3t.rearrange("p (k m) -> p k m", k=K2)

        def load_x(off, cols, interleave_w1=False):
            xsb = xp.tile([128, KF, BS], BF16, tag="xsb")
            xl = xp.tile([128, BS], BF16, tag="xl")
            bsl = slice(off, off + cols)
            if interleave_w1:
                # startup: alternate x k-tiles with w1 k-rows so the first
                # matmul chain can start ~2.5us in instead of after all of w1
                # weights go out on the ACT HWDGE ring, x on the sync ring:
                # the two streams run concurrently at startup
                for k in range(KF):
                    nc.sync.dma_start(xsb[:, k, :cols], xt_r[:, k, bsl])
                    nc.scalar.dma_start(w1sb[:, k, :], w1t[k * 128 : (k + 1) * 128, :])
                nc.sync.dma_start(xl[:, :cols], xlt[:, bsl])
                nc.scalar.dma_start(w1lsb[:], w1lt[:])
                nc.scalar.dma_start(g1sb[:], g1[:])
            else:
                nc.sync.dma_start(xsb[:, :, :cols], xt_r[:, :, bsl])
                # leftover replicas ride the ACT ring: keeps the sync ring
                # 100% on the main x stream during the startup backlog
                nc.scalar.dma_start(xl[:, :cols], xlt[:, bsl])
            return xsb, xl

        def load_l23_weights():
            nc.scalar.dma_start(w2sb[:], w2t_r[:])
            nc.scalar.dma_start(w3sb[:], w3t_r[:])
            nc.scalar.dma_start(g2sb[:], g2[:])

        def sq_tree(hraw, cols, tag):
            """Square + pairwise reduce over the 8 feature tiles -> octs."""
            hsq = hsp.tile([128, HO, BS], BF16, tag="hsq" + tag)
            pairs = hsp.tile([128, HO // 2, BS], BF16, tag="hsqp" + tag)
            quads = hsp.tile([128, 2, BS], BF16, tag="hsqq" + tag)
            octs = hsp.tile([128, BS], BF16, tag="hsqo" + tag)
            nc.vector.tensor_mul(hsq[:, :, :cols], hraw[:, :, :cols], hraw[:, :, :cols])
            ev = hsq[:, :, :cols].rearrange("p (j two) f -> p two j f", two=2)
            nc.vector.tensor_add(pairs[:, :, :cols], ev[:, 0], ev[:, 1])
            nc.vector.tensor_add(
                quads[:, :, :cols], pairs[:, 0:2, :cols], pairs[:, 2:4, :cols]
            )
            nc.vector.tensor_add(octs[:, :cols], quads[:, 0, :cols], quads[:, 1, :cols])
            return octs

        def l1_mm(xsb, xl, cols):
            """L1 matmul: two waves of 4 output tiles; each wave ends with 4
            concurrent row-packed 32-row matmuls for the 16 leftover
            features."""
            hraw = hp.tile([128, HO, BS], BF16, tag="hraw1")
            for half in range(2):
                pss = []
                for g_ in range(4):
                    oi = 4 * half + g_
                    ps = pp.tile([128, BS], F32, tag="mm")
                    pss.append(ps)
                    for k in range(KF):
                        nc.tensor.matmul(
                            ps[:, :cols],
                            lhsT=w1sb[:, k, oi * 128 : (oi + 1) * 128],
                            rhs=xsb[:, k, :cols],
                            start=(k == 0),
                            stop=False,
                        )
                for g_ in range(4):
                    nc.tensor.matmul(
                        pss[g_][:, :cols],
                        lhsT=w1lsb[32 * g_ : 32 * g_ + 32, half * 128 : (half + 1) * 128],
                        rhs=xl[32 * g_ : 32 * g_ + 32, :cols],
                        start=False,
                        stop=True,
                        tile_position=(32 * g_, 0),
                        skip_group_check=True,
                    )
                for g_ in range(4):
                    oi = 4 * half + g_
                    nc.scalar.copy(hraw[:, oi, :cols], pss[g_][:, :cols])
            octs = sq_tree(hraw, cols, "1")
            return hraw, octs

        def l2_mm_half(h1, cols, half, fine=False, tiles=None):
            """One half (4 output tiles) of the L2 matmul block.  The caller
            emits the two halves around norm-L1, which puts the L1
            ones-matmul mid-block (its reduce-tree inputs are ready by then,
            so no PE stall) and gets the rsqrt chain onto DVE early."""
            if half == 0:
                hraw = hp.tile([128, HO, BS], BF16, tag="hraw2")
                if fine:
                    hsq = hsp.tile([128, HO, BS], BF16, tag="hsq2")
                    pairs = hsp.tile([128, HO // 2, BS], BF16, tag="hsqp2")
                    quads = hsp.tile([128, 2, BS], BF16, tag="hsqq2")
                    octs = hsp.tile([128, BS], BF16, tag="hsqo2")
                    tiles = (hraw, hsq, pairs, quads, octs)
                else:
                    tiles = (hraw, None, None, None, None)
            hraw, hsq, pairs, quads, octs = tiles
            for oi in range(4 * half, 4 * half + 4):
                ps = pp.tile([128, BS], F32, tag="mm")
                for k in range(K2):
                    nc.tensor.matmul(
                        ps[:, :cols],
                        lhsT=w2sb[:, k, oi * 128 : (oi + 1) * 128],
                        rhs=h1[:, k, :cols],
                        start=(k == 0),
                        stop=(k == K2 - 1),
                    )
                nc.scalar.copy(hraw[:, oi, :cols], ps[:, :cols])
                if fine:
                    nc.vector.tensor_mul(
                        hsq[:, oi, :cols], hraw[:, oi, :cols], hraw[:, oi, :cols]
                    )
                    if oi % 2 == 1:
                        nc.vector.tensor_add(
                            pairs[:, oi // 2, :cols],
                            hsq[:, oi - 1, :cols],
                            hsq[:, oi, :cols],
                        )
            if half == 1 and fine:
                nc.vector.tensor_add(
                    quads[:, :, :cols], pairs[:, 0:2, :cols], pairs[:, 2:4, :cols]
                )
                nc.vector.tensor_add(
                    octs[:, :cols], quads[:, 0, :cols], quads[:, 1, :cols]
                )
            return tiles

        def norm(state, g_sb, pool, tag, cols, fine=False):
            """ones-matmul partition reduce, rsqrt via magic seed + 1 Newton
            step, then per-tile prescale (in-place) + gelu."""
            hraw, octs = state
            hout = pool.tile([128, HO, BS], BF16, tag="hout" + tag)
            ssq = sp.tile([128, BS], F32, tag="ssq")
            nc.tensor.matmul(
                ssq[:, :cols], lhsT=ones[:], rhs=octs[:, :cols], start=True, stop=True
            )
            ti = rp.tile([128, BS], I32, tag="ti" + tag)
            nc.vector.tensor_scalar(
                ti[:, :cols], ssq[:, :cols].bitcast(I32), 1, -1,
                op0=ALU.arith_shift_right, op1=ALU.bitwise_xor,
            )  # ~(v >> 1)
            y0 = rp.tile([128, BS], I32, tag="y0" + tag)
            nc.vector.tensor_scalar(y0[:, :cols], ti[:, :cols], MAGIC + 1, None, op0=ALU.add)
            y0f = y0[:, :cols].bitcast(F32)
            t1 = rp.tile([128, BS], F32, tag="t1" + tag)
            nc.vector.tensor_mul(t1[:, :cols], y0f, y0f)
            t2 = rp.tile([128, BS], F32, tag="t2" + tag)
            nc.vector.tensor_mul(t2[:, :cols], t1[:, :cols], ssq[:, :cols])
            nc.vector.tensor_scalar(
                t2[:, :cols], t2[:, :cols], -0.5, 1.5, op0=ALU.mult, op1=ALU.add
            )
            rinv = rp.tile([128, BS], BF16, tag="rinv" + tag)
            nc.vector.tensor_mul(rinv[:, :cols], y0f, t2[:, :cols])
            # prescale per output tile (both operands dense step-1 bf16 ->
            # DVE 2x mode; the [128,HO,BS]-with-broadcast form runs at 1x),
            # writing in place into hraw
            for oi in range(HO):
                nc.vector.tensor_mul(
                    hraw[:, oi, :cols], hraw[:, oi, :cols], rinv[:, :cols]
                )
                if fine:
                    if g_is_one:
                        nc.scalar.activation(
                            hout[:, oi, :cols], hraw[:, oi, :cols], ACTF.Gelu
                        )
                    else:
                        nc.scalar.activation(
                            hout[:, oi, :cols], hraw[:, oi, :cols], ACTF.Gelu,
                            scale=g_sb[:, oi : oi + 1],
                        )
            if not fine:
                if g_is_one:
                    nc.scalar.activation(hout[:, :, :cols], hraw[:, :, :cols], ACTF.Gelu)
                else:
                    for oi in range(HO):
                        nc.scalar.activation(
                            hout[:, oi, :cols], hraw[:, oi, :cols], ACTF.Gelu,
                            scale=g_sb[:, oi : oi + 1],
                        )
            return hout

        def l3(h2, off, cols):
            # L3 (M=10): pack 4 col-strips of the PE array concurrently,
            # 2 K-chunks accumulated per strip; strips merge on ACT+DVE.
            ps3 = p3.tile([128, BS], F32, tag="mm3")
            for g_ in range(4):
                for kk in range(2):
                    k = 2 * g_ + kk
                    nc.tensor.matmul(
                        ps3[32 * g_ : 32 * g_ + OUT, :cols],
                        lhsT=w3sb[:, k, :],
                        rhs=h2[:, k, :cols],
                        start=(kk == 0),
                        stop=(kk == 1),
                        tile_position=(0, 32 * g_),
                    )
            osb = op.tile([OUT, BS], F32, tag="osb")
            nc.scalar.copy(osb[:, :cols], ps3[0:OUT, :cols])
            for g_ in range(1, 4):
                nc.vector.tensor_add(
                    osb[:, :cols], osb[:, :cols], ps3[32 * g_ : 32 * g_ + OUT, :cols]
                )
            nc.sync.dma_start(outt[:, off : off + cols], osb[:, :cols])

        def norm2_head(state, cols, sfx):
            """Drain: ones-matmul + rsqrt chain for one item.  Emitted for
            BOTH drain items before either tail, so the second item's chain
            pipelines behind the first instead of queueing after its strip
            matmuls."""
            hraw, octs = state
            ssq = sp.tile([128, BS], F32, tag="ssq")
            nc.tensor.matmul(
                ssq[:, :cols], lhsT=ones[:], rhs=octs[:, :cols], start=True, stop=True
            )
            ti = rp.tile([128, BS], I32, tag="ti2")
            nc.vector.tensor_scalar(
                ti[:, :cols], ssq[:, :cols].bitcast(I32), 1, -1,
                op0=ALU.arith_shift_right, op1=ALU.bitwise_xor,
            )
            y0 = rp.tile([128, BS], I32, tag="y02")
            nc.vector.tensor_scalar(y0[:, :cols], ti[:, :cols], MAGIC + 1, None, op0=ALU.add)
            y0f = y0[:, :cols].bitcast(F32)
            t1 = rp.tile([128, BS], F32, tag="t12")
            nc.vector.tensor_mul(t1[:, :cols], y0f, y0f)
            t2 = rp.tile([128, BS], F32, tag="t22")
            nc.vector.tensor_mul(t2[:, :cols], t1[:, :cols], ssq[:, :cols])
            nc.vector.tensor_scalar(
                t2[:, :cols], t2[:, :cols], -0.5, 1.5, op0=ALU.mult, op1=ALU.add
            )
            rinv = rp.tile([128, BS], BF16, tag="rinv2" + sfx)
            nc.vector.tensor_mul(rinv[:, :cols], y0f, t2[:, :cols])
            return hraw, rinv

        def norm2_l3_tail(head, off, cols):
            hraw, rinv = head
            hout = hq2.tile([128, HO, BS], BF16, tag="hout2")
            ps3 = p3.tile([128, BS], F32, tag="mm3")
            for oi in range(HO):
                nc.vector.tensor_mul(
                    hraw[:, oi, :cols], hraw[:, oi, :cols], rinv[:, :cols]
                )
                if g_is_one:
                    nc.scalar.activation(
                        hout[:, oi, :cols], hraw[:, oi, :cols], ACTF.Gelu
                    )
                else:
                    nc.scalar.activation(
                        hout[:, oi, :cols], hraw[:, oi, :cols], ACTF.Gelu,
                        scale=g2sb[:, oi : oi + 1],
                    )
                if oi % 2 == 1:
                    g_ = oi // 2
                    filler(2)
                    for kk in range(2):
                        k = 2 * g_ + kk
                        nc.tensor.matmul(
                            ps3[32 * g_ : 32 * g_ + OUT, :cols],
                            lhsT=w3sb[:, k, :],
                            rhs=hout[:, k, :cols],
                            start=(kk == 0),
                            stop=(kk == 1),
                            tile_position=(0, 32 * g_),
                        )
            filler(3)
            osb = op.tile([OUT, BS], F32, tag="osb")
            nc.scalar.copy(osb[:, :cols], ps3[0:OUT, :cols])
            for g_ in range(1, 4):
                nc.vector.tensor_add(
                    osb[:, :cols], osb[:, :cols], ps3[32 * g_ : 32 * g_ + OUT, :cols]
                )
            nc.sync.dma_start(outt[:, off : off + cols], osb[:, :cols])
            filler(3)

        def filler(n):
            # tiny no-dependency matmuls sprinkled through the drain: they
            # execute the moment PE reaches them, keeping the HAM activity
            # window non-idle so the PE clock stays at 2.4 GHz while real
            # work waits on norm chains
            fil = pp.tile([128, BS], F32, tag="mm")
            for i in range(n):
                nc.tensor.matmul(
                    fil[0:32, 0:128],
                    lhsT=ones[:, 0:32],
                    rhs=ones[:],
                    start=True,
                    stop=True,
                    skip_group_check=True,
                )

        nc.vector.memset(ones[:], 1.0 / H)

        # software pipeline over items; superstep s emits
        #   [L1(s)] [norm L2(s-2)] [L2(s-1)] [norm L1(s)] [L3(s-2)]
        # so each ones-matmul sits behind a full matmul block (its DVE
        # reduce-tree inputs are long since ready -> no PE stall), and the
        # L3 of item s-2 runs after norm L2(s-2) completed mid-superstep.
        st1: dict[int, object] = {}
        st2: dict[int, object] = {}
        h1s: dict[int, object] = {}
        h2s: dict[int, object] = {}
        xq: dict[int, tuple] = {}
        for s in range(NI + 2):
            if s == 0:
                xq[0] = load_x(*items[0], interleave_w1=True)
            if s < NI:
                off, cols = items[s]
                st1[s] = l1_mm(*xq.pop(s), cols)
            if s + 1 < NI:
                # prefetch next item's x now, so in the DMA queue it sits
                # before this superstep's other loads (w2/w3 at s==0)
                xq[s + 1] = load_x(*items[s + 1])
            if s == 0:
                load_l23_weights()
            if 2 <= s and (s - 2) not in FINE:
                c = s - 2
                h2s[c] = norm(st2.pop(c), g2sb, hq2, "2", items[c][1])
            if 1 <= s <= NI:
                c = s - 1
                fine2 = c in FINE
                if s >= NI - 1:
                    filler(3)
                t2 = l2_mm_half(h1s[c], items[c][1], 0, fine=fine2)
            if s < NI:
                h1s[s] = norm(st1.pop(s), g1sb, hq1, "1", cols, fine=(s in FINE))
            if 1 <= s <= NI:
                c = s - 1
                t2 = l2_mm_half(h1s.pop(c), items[c][1], 1, fine=fine2, tiles=t2)
                if fine2:
                    st2[c] = (t2[0], t2[4])
                else:
                    # L2 square/reduce tree emitted AFTER norm L1's DVE chain:
                    # keeps rsqrt->prescale->gelu to h1out off the DVE
                    # critical path (the L2 ones-matmul only fires next
                    # superstep)
                    st2[c] = (t2[0], sq_tree(t2[0], items[c][1], "2"))
            if 2 <= s:
                c = s - 2
                if c in FINE:
                    if c == NI - 2:
                        # drain: both items' ones+rsqrt heads first, then
                        # both prescale/gelu/strip tails -- the second
                        # chain's PE/DVE work pipelines behind the first
                        # instead of queueing after its strip matmuls
                        filler(3)
                        ha = norm2_head(st2.pop(NI - 2), items[NI - 2][1], "a")
                        hb = norm2_head(st2.pop(NI - 1), items[NI - 1][1], "b")
                        norm2_l3_tail(ha, items[NI - 2][0], items[NI - 2][1])
                        norm2_l3_tail(hb, items[NI - 1][0], items[NI - 1][1])
                else:
                    l3(h2s.pop(c), items[c][0], items[c][1])

    nc.compile()
    return nc


def _quant(w):
    s = max(float(np.mean(np.abs(w))), EPS_Q)
    return np.clip(np.round(w / s), -1.0, 1.0)


def kernel(x, w1, g1, w2, g2, w3):
    global LAST_RESULTS
    bf = ml_dtypes.bfloat16

    w1q = _quant(np.asarray(w1, np.float32))  # [H, IN]
    w2q = _quant(np.asarray(w2, np.float32))  # [H, H]
    w3q = _quant(np.asarray(w3, np.float32))  # [OUT, H]

    f8 = ml_dtypes.float8_e4m3fn  # ternary weights are exact in e4m3
    w1T = w1q.T.astype(f8)  # [IN, H]
    w1t_np = np.ascontiguousarray(w1T[: KF * 128])
    w1lt_np = np.zeros([128, 2 * 128], dtype=f8)
    for j in range(2):
        for g_ in range(4):
            oi = 4 * j + g_
            w1lt_np[32 * g_ : 32 * g_ + KL, j * 128 : (j + 1) * 128] = w1T[
                KF * 128 :, oi * 128 : (oi + 1) * 128
            ]
    w2t_np = np.ascontiguousarray(w2q.T.astype(f8))
    # w3 packed so each partition's row is contiguous: [128, K2*OUT]
    w3t_np = np.ascontiguousarray(
        w3q.T.astype(f8).reshape(K2, 128, OUT).transpose(1, 0, 2).reshape(128, K2 * OUT)
    )
    g1_np = np.ascontiguousarray(np.asarray(g1, np.float32).reshape(HO, 128).T)
    g2_np = np.ascontiguousarray(np.asarray(g2, np.float32).reshape(HO, 128).T)

    xT = np.asarray(x, np.float32).T.astype(bf)  # [IN, B]
    xt_np = np.ascontiguousarray(xT[: KF * 128])
    xlt_np = np.zeros([128, B], dtype=bf)
    for g_ in range(4):
        xlt_np[32 * g_ : 32 * g_ + KL] = xT[KF * 128 :]

    g_is_one = bool(np.all(np.asarray(g1) == 1.0) and np.all(np.asarray(g2) == 1.0))
    key = ("nc", g_is_one)
    if key not in _cache:
        _cache[key] = _build(g_is_one)
    nc = _cache[key]

    in_maps = []
    for i in range(N_CORES):
        sl = slice(i * BPC, (i + 1) * BPC)
        in_maps.append(
            {
                "xt": np.ascontiguousarray(xt_np[:, sl]),
                "xlt": np.ascontiguousarray(xlt_np[:, sl]),
                "w1t": w1t_np,
                "w1lt": w1lt_np,
                "w2t": w2t_np,
                "w3t": w3t_np,
                "g1": g1_np,
                "g2": g2_np,
            }
        )

    res = run_bass_kernel_spmd(nc, in_maps, core_ids=list(range(N_CORES)))
    LAST_RESULTS = res

    out = np.empty([B, OUT], dtype=np.float32)
    for i in range(N_CORES):
        out[i * BPC : (i + 1) * BPC] = res.results[i]["outt"].T
    return out


# revision 29
# speedup vs baseline: 1.0097x; 1.0097x over previous
"""BitNet MNIST MLP forward on 8 Trainium2 NeuronCores (pure data parallel).

Reference math (per _bitlinear): out = (x/sx) @ w_q.T * sx with per-row
sx = max(|x|) -- the activation scale cancels exactly, so we compute
x @ w_q.T directly.  Ternary w_q is precomputed on host (exact in bf16).

Per-core dataflow (batch shard 8192 rows; work items of 512 batch cols,
with the first and last two chunks split into 256-col halves to shorten
the pipeline fill/drain dependency chains):
  activations live feature-major [feat_part(128) x batch_free] in SBUF, so
  every layer's matmul contracts features on partitions with stationary
  (pre-transposed) weights and NO on-chip transposes.  Ternary weights
  ship as fp8e4 (exact for {-1,0,+1}): halves weight DMA and gets the 4x
  fast-weight-load path.  Weight DMAs ride the ACT HWDGE ring, x the sync
  ring -- the two streams run concurrently through the startup backlog.
  L1 contraction: 6 full 128-row k-tiles + a 16-row leftover (784 = 6*128
  + 16).  The leftover is NOT padded to a full tile: the 16 rows are
  replicated host-side at partition offsets 0/32/64/96 and contracted by
  4 concurrent 32-row matmuls (tile_position row packing), saving 6 of 8
  full-cost matmul slots per chunk.
  RMS mean(h^2) over the 1024 features = ones(1/1024)-matmul accumulated
  over the 8 feature tiles -> replicated [128, BS] PSUM value.  Each
  ones-matmul is emitted AFTER a following matmul block so the PE never
  stalls on the DVE square/reduce tree; the L2 block is emitted in two
  halves around norm-L1 so the L1 ones-matmul fires mid-block and the
  rsqrt chain reaches DVE with slack.  The L2 square-tree is emitted
  after norm-L1's chain (off the h1out critical path).
  rsqrt = int bit-trick seed + 1 Newton iteration on DVE.
  gelu(h*rinv*g): rinv via per-tile DVE tensor-tensor (2x mode, in-place
  into hraw), g via per-partition ACT scale.  Superstep order per item s:
    [L1(s) mm] [norm L2(s-2)] [L2(s-1) mm 1st half] [norm L1(s)]
    [L2(s-1) mm 2nd half] [L2 tree] [L3(s-2)]
  Drain (last two items): fused fine norm2+L3 (each L3 col-strip fires as
  soon as its two gelu tiles land) plus tiny no-dep filler matmuls that
  keep the HAM activity window busy so the PE clock stays at 2.4 GHz.
"""

import os
from contextlib import ExitStack

import numpy as np
import ml_dtypes

import concourse.bacc as bacc
import concourse.bass as bass
import concourse.mybir as mybir
import concourse.tile as tile
from concourse.bass_utils import run_bass_kernel_spmd

N_CORES = 8
B, IN, H, OUT = 65536, 784, 1024, 10
BPC = B // N_CORES  # 8192 rows per core
KF = 6              # full 128-row contraction tiles of x (768 features)
KL = IN - KF * 128  # 16 leftover features
K2 = H // 128       # 8 contraction tiles, layers 2/3
HO = H // 128       # 8 output-feature tiles
BS = 512            # batch columns per full chunk
NB = BPC // BS      # 16 full chunks
EPS_Q = 1e-5
MAGIC = 0x5F3759DF

F32 = mybir.dt.float32
BF16 = mybir.dt.bfloat16
FP8 = mybir.dt.float8e4  # ternary weights are exact in e4m3
I32 = mybir.dt.int32
ALU = mybir.AluOpType
ACTF = mybir.ActivationFunctionType

_cache = {}
LAST_RESULTS = None  # test.py reads exec_time_ns off this


def _build(g_is_one=True):
    # Bacc (not raw Bass): its compile() runs generate_event_semaphores(),
    # which splits multi-wait sync_infos down to the 1-wait HW limit.
    nc = bacc.Bacc("TRN2", target_bir_lowering=False, debug=False, num_devices=N_CORES)

    xt = nc.dram_tensor("xt", [KF * 128, BPC], BF16, kind="ExternalInput").ap()
    xlt = nc.dram_tensor("xlt", [128, BPC], BF16, kind="ExternalInput").ap()
    w1t = nc.dram_tensor("w1t", [KF * 128, H], FP8, kind="ExternalInput").ap()
    w1lt = nc.dram_tensor("w1lt", [128, 2 * 128], FP8, kind="ExternalInput").ap()
    w2t = nc.dram_tensor("w2t", [H, H], FP8, kind="ExternalInput").ap()
    w3t = nc.dram_tensor("w3t", [128, K2 * OUT], FP8, kind="ExternalInput").ap()
    g1 = nc.dram_tensor("g1", [128, HO], F32, kind="ExternalInput").ap()
    g2 = nc.dram_tensor("g2", [128, HO], F32, kind="ExternalInput").ap()
    outt = nc.dram_tensor("outt", [OUT, BPC], F32, kind="ExternalOutput").ap()

    # work items: the first and last two chunks are split into 256-col
    # halves (shorter dependency chains through pipeline fill and drain;
    # fine-mode norm there), the middle 13 chunks are full 512-col items.
    items = [(0, BS // 2), (BS // 2, BS // 2)]
    items += [(c * BS, BS) for c in range(1, NB - 2)]
    for c in (NB - 2, NB - 1):
        items += [(c * BS, BS // 2), (c * BS + BS // 2, BS // 2)]
    NI = len(items)
    # fine/fused mode only for the last two items (the exposed drain)
    FINE = {NI - 2, NI - 1}

    with tile.TileContext(nc) as tc, ExitStack() as ctx:
        wp = ctx.enter_context(tc.tile_pool(name="weights", bufs=1))
        xp = ctx.enter_context(tc.tile_pool(name="x", bufs=3))
        hp = ctx.enter_context(tc.tile_pool(name="h", bufs=3))
        hsp = ctx.enter_context(tc.tile_pool(name="hsq", bufs=1))
        hq1 = ctx.enter_context(tc.tile_pool(name="h1out", bufs=2))
        hq2 = ctx.enter_context(tc.tile_pool(name="h2out", bufs=2))
        rp = ctx.enter_context(tc.tile_pool(name="rsq", bufs=1))
        op = ctx.enter_context(tc.tile_pool(name="out", bufs=3))
        pp = ctx.enter_context(tc.tile_pool(name="ps", bufs=5, space="PSUM"))
        sp = ctx.enter_context(tc.tile_pool(name="ssq", bufs=1, space="PSUM"))
        p3 = ctx.enter_context(tc.tile_pool(name="ps3", bufs=2, space="PSUM"))

        w1sb = wp.tile([128, KF, H], FP8)
        w1lsb = wp.tile([128, 2 * 128], FP8)
        g1sb = wp.tile([128, HO], F32)
        ones = wp.tile([128, 128], BF16)
        w2sb = wp.tile([128, K2, H], FP8)
        w3sb = wp.tile([128, K2, OUT], FP8)
        g2sb = wp.tile([128, HO], F32)

        xt_r = xt.rearrange("(k p) b -> p k b", p=128)
        w2t_r = w2t.rearrange("(k p) h -> p k h", p=128)
        w3t_r = w3t.rearrange("p (k m) -> p k m", k=K2)

        def load_x(off, cols, interleave_w1=False):
            xsb = xp.tile([128, KF, BS], BF16, tag="xsb")
            xl = xp.tile([128, BS], BF16, tag="xl")
            bsl = slice(off, off + cols)
            if interleave_w1:
                # startup: alternate x k-tiles with w1 k-rows so the first
                # matmul chain can start ~2.5us in instead of after all of w1
                # weights go out on the ACT HWDGE ring, x on the sync ring:
                # the two streams run concurrently at startup
                for k in range(KF):
                    nc.sync.dma_start(xsb[:, k, :cols], xt_r[:, k, bsl])
                    nc.scalar.dma_start(w1sb[:, k, :], w1t[k * 128 : (k + 1) * 128, :])
                nc.sync.dma_start(xl[:, :cols], xlt[:, bsl])
                nc.scalar.dma_start(w1lsb[:], w1lt[:])
                nc.scalar.dma_start(g1sb[:], g1[:])
            else:
                nc.sync.dma_start(xsb[:, :, :cols], xt_r[:, :, bsl])
                # leftover replicas ride the ACT ring: keeps the sync ring
                # 100% on the main x stream during the startup backlog
                nc.scalar.dma_start(xl[:, :cols], xlt[:, bsl])
            return xsb, xl

        def load_l23_weights():
            nc.scalar.dma_start(w2sb[:], w2t_r[:])
            nc.scalar.dma_start(w3sb[:], w3t_r[:])
            nc.scalar.dma_start(g2sb[:], g2[:])

        def sq_tree(hraw, cols, tag):
            """Square + pairwise reduce over the 8 feature tiles -> octs."""
            hsq = hsp.tile([128, HO, BS], BF16, tag="hsq" + tag)
            pairs = hsp.tile([128, HO // 2, BS], BF16, tag="hsqp" + tag)
            quads = hsp.tile([128, 2, BS], BF16, tag="hsqq" + tag)
            octs = hsp.tile([128, BS], BF16, tag="hsqo" + tag)
            nc.vector.tensor_mul(hsq[:, :, :cols], hraw[:, :, :cols], hraw[:, :, :cols])
            ev = hsq[:, :, :cols].rearrange("p (j two) f -> p two j f", two=2)
            nc.vector.tensor_add(pairs[:, :, :cols], ev[:, 0], ev[:, 1])
            nc.vector.tensor_add(
                quads[:, :, :cols], pairs[:, 0:2, :cols], pairs[:, 2:4, :cols]
            )
            nc.vector.tensor_add(octs[:, :cols], quads[:, 0, :cols], quads[:, 1, :cols])
            return octs

        def l1_mm(xsb, xl, cols):
            """L1 matmul: two waves of 4 output tiles; each wave ends with 4
            concurrent row-packed 32-row matmuls for the 16 leftover
            features."""
            hraw = hp.tile([128, HO, BS], BF16, tag="hraw1")
            for half in range(2):
                pss = []
                for g_ in range(4):
                    oi = 4 * half + g_
                    ps = pp.tile([128, BS], F32, tag="mm")
                    pss.append(ps)
                    for k in range(KF):
                        nc.tensor.matmul(
                            ps[:, :cols],
                            lhsT=w1sb[:, k, oi * 128 : (oi + 1) * 128],
                            rhs=xsb[:, k, :cols],
                            start=(k == 0),
                            stop=False,
                        )
                for g_ in range(4):
                    nc.tensor.matmul(
                        pss[g_][:, :cols],
                        lhsT=w1lsb[32 * g_ : 32 * g_ + 32, half * 128 : (half + 1) * 128],
                        rhs=xl[32 * g_ : 32 * g_ + 32, :cols],
                        start=False,
                        stop=True,
                        tile_position=(32 * g_, 0),
                        skip_group_check=True,
                    )
                for g_ in range(4):
                    oi = 4 * half + g_
                    nc.scalar.copy(hraw[:, oi, :cols], pss[g_][:, :cols])
            octs = sq_tree(hraw, cols, "1")
            return hraw, octs

        def l2_mm_half(h1, cols, half, fine=False, tiles=None):
            """One half (4 output tiles) of the L2 matmul block.  The caller
            emits the two halves around norm-L1, which puts the L1
            ones-matmul mid-block (its reduce-tree inputs are ready by then,
            so no PE stall) and gets the rsqrt chain onto DVE early."""
            if half == 0:
                hraw = hp.tile([128, HO, BS], BF16, tag="hraw2")
                if fine:
                    hsq = hsp.tile([128, HO, BS], BF16, tag="hsq2")
                    pairs = hsp.tile([128, HO // 2, BS], BF16, tag="hsqp2")
                    quads = hsp.tile([128, 2, BS], BF16, tag="hsqq2")
                    octs = hsp.tile([128, BS], BF16, tag="hsqo2")
                    tiles = (hraw, hsq, pairs, quads, octs)
                else:
                    tiles = (hraw, None, None, None, None)
            hraw, hsq, pairs, quads, octs = tiles
            for oi in range(4 * half, 4 * half + 4):
                ps = pp.tile([128, BS], F32, tag="mm")
                for k in range(K2):
                    nc.tensor.matmul(
                        ps[:, :cols],
                        lhsT=w2sb[:, k, oi * 128 : (oi + 1) * 128],
                        rhs=h1[:, k, :cols],
                        start=(k == 0),
                        stop=(k == K2 - 1),
                    )
                nc.scalar.copy(hraw[:, oi, :cols], ps[:, :cols])
                if fine:
                    nc.vector.tensor_mul(
                        hsq[:, oi, :cols], hraw[:, oi, :cols], hraw[:, oi, :cols]
                    )
                    if oi % 2 == 1:
                        nc.vector.tensor_add(
                            pairs[:, oi // 2, :cols],
                            hsq[:, oi - 1, :cols],
                            hsq[:, oi, :cols],
                        )
            if half == 1 and fine:
                nc.vector.tensor_add(
                    quads[:, :, :cols], pairs[:, 0:2, :cols], pairs[:, 2:4, :cols]
                )
                nc.vector.tensor_add(
                    octs[:, :cols], quads[:, 0, :cols], quads[:, 1, :cols]
                )
            return tiles

        def norm(state, g_sb, pool, tag, cols, fine=False):
            """ones-matmul partition reduce, rsqrt via magic seed + 1 Newton
            step, then per-tile prescale (in-place) + gelu."""
            hraw, octs = state
            hout = pool.tile([128, HO, BS], BF16, tag="hout" + tag)
            ssq = sp.tile([128, BS], F32, tag="ssq")
            nc.tensor.matmul(
                ssq[:, :cols], lhsT=ones[:], rhs=octs[:, :cols], start=True, stop=True
            )
            ti = rp.tile([128, BS], I32, tag="ti" + tag)
            nc.vector.tensor_scalar(
                ti[:, :cols], ssq[:, :cols].bitcast(I32), 1, -1,
                op0=ALU.arith_shift_right, op1=ALU.bitwise_xor,
            )  # ~(v >> 1)
            y0 = rp.tile([128, BS], I32, tag="y0" + tag)
            nc.vector.tensor_scalar(y0[:, :cols], ti[:, :cols], MAGIC + 1, None, op0=ALU.add)
            y0f = y0[:, :cols].bitcast(F32)
            t1 = rp.tile([128, BS], F32, tag="t1" + tag)
            nc.vector.tensor_mul(t1[:, :cols], y0f, y0f)
            t2 = rp.tile([128, BS], F32, tag="t2" + tag)
            nc.vector.tensor_mul(t2[:, :cols], t1[:, :cols], ssq[:, :cols])
            nc.vector.tensor_scalar(
                t2[:, :cols], t2[:, :cols], -0.5, 1.5, op0=ALU.mult, op1=ALU.add
            )
            rinv = rp.tile([128, BS], BF16, tag="rinv" + tag)
            nc.vector.tensor_mul(rinv[:, :cols], y0f, t2[:, :cols])
            # prescale per output tile (both operands dense step-1 bf16 ->
            # DVE 2x mode; the [128,HO,BS]-with-broadcast form runs at 1x),
            # writing in place into hraw
            for oi in range(HO):
                nc.vector.tensor_mul(
                    hraw[:, oi, :cols], hraw[:, oi, :cols], rinv[:, :cols]
                )
                if fine:
                    if g_is_one:
                        nc.scalar.activation(
                            hout[:, oi, :cols], hraw[:, oi, :cols], ACTF.Gelu
                        )
                    else:
                        nc.scalar.activation(
                            hout[:, oi, :cols], hraw[:, oi, :cols], ACTF.Gelu,
                            scale=g_sb[:, oi : oi + 1],
                        )
            if not fine:
                if g_is_one:
                    nc.scalar.activation(hout[:, :, :cols], hraw[:, :, :cols], ACTF.Gelu)
                else:
                    for oi in range(HO):
                        nc.scalar.activation(
                            hout[:, oi, :cols], hraw[:, oi, :cols], ACTF.Gelu,
                            scale=g_sb[:, oi : oi + 1],
                        )
            return hout

        def l3(h2, off, cols):
            # L3 (M=10): pack 4 col-strips of the PE array concurrently,
            # 2 K-chunks accumulated per strip; strips merge on ACT+DVE.
            ps3 = p3.tile([128, BS], F32, tag="mm3")
            for g_ in range(4):
                for kk in range(2):
                    k = 2 * g_ + kk
                    nc.tensor.matmul(
                        ps3[32 * g_ : 32 * g_ + OUT, :cols],
                        lhsT=w3sb[:, k, :],
                        rhs=h2[:, k, :cols],
                        start=(kk == 0),
                        stop=(kk == 1),
                        tile_position=(0, 32 * g_),
                    )
            osb = op.tile([OUT, BS], F32, tag="osb")
            nc.scalar.copy(osb[:, :cols], ps3[0:OUT, :cols])
            for g_ in range(1, 4):
                nc.vector.tensor_add(
                    osb[:, :cols], osb[:, :cols], ps3[32 * g_ : 32 * g_ + OUT, :cols]
                )
            nc.sync.dma_start(outt[:, off : off + cols], osb[:, :cols])

        def norm2_l3_fused(state, off, cols):
            """Drain path for the last items: norm-L2 in fine mode with the
            L3 col-strip matmuls fired as soon as their two gelu tiles land,
            collapsing the norm2->L3 serial chain."""
            hraw, octs = state
            hout = hq2.tile([128, HO, BS], BF16, tag="hout2")
            ssq = sp.tile([128, BS], F32, tag="ssq")
            nc.tensor.matmul(
                ssq[:, :cols], lhsT=ones[:], rhs=octs[:, :cols], start=True, stop=True
            )
            ti = rp.tile([128, BS], I32, tag="ti2")
            nc.vector.tensor_scalar(
                ti[:, :cols], ssq[:, :cols].bitcast(I32), 1, -1,
                op0=ALU.arith_shift_right, op1=ALU.bitwise_xor,
            )
            y0 = rp.tile([128, BS], I32, tag="y02")
            nc.vector.tensor_scalar(y0[:, :cols], ti[:, :cols], MAGIC + 1, None, op0=ALU.add)
            y0f = y0[:, :cols].bitcast(F32)
            t1 = rp.tile([128, BS], F32, tag="t12")
            nc.vector.tensor_mul(t1[:, :cols], y0f, y0f)
            t2 = rp.tile([128, BS], F32, tag="t22")
            nc.vector.tensor_mul(t2[:, :cols], t1[:, :cols], ssq[:, :cols])
            nc.vector.tensor_scalar(
                t2[:, :cols], t2[:, :cols], -0.5, 1.5, op0=ALU.mult, op1=ALU.add
            )
            rinv = rp.tile([128, BS], BF16, tag="rinv2")
            nc.vector.tensor_mul(rinv[:, :cols], y0f, t2[:, :cols])
            ps3 = p3.tile([128, BS], F32, tag="mm3")
            for oi in range(HO):
                nc.vector.tensor_mul(
                    hraw[:, oi, :cols], hraw[:, oi, :cols], rinv[:, :cols]
                )
                if g_is_one:
                    nc.scalar.activation(
                        hout[:, oi, :cols], hraw[:, oi, :cols], ACTF.Gelu
                    )
                else:
                    nc.scalar.activation(
                        hout[:, oi, :cols], hraw[:, oi, :cols], ACTF.Gelu,
                        scale=g2sb[:, oi : oi + 1],
                    )
                if oi % 2 == 1:
                    g_ = oi // 2
                    filler(2)
                    for kk in range(2):
                        k = 2 * g_ + kk
                        # all 8 k-tiles accumulate into ONE psum strip: the
                        # serialized matmuls cost ~0.4us of idle drain PE but
                        # drop the 3-add + copy merge from the final chain
                        nc.tensor.matmul(
                            ps3[0:OUT, :cols],
                            lhsT=w3sb[:, k, :],
                            rhs=hout[:, k, :cols],
                            start=(oi == 1 and kk == 0),
                            stop=(oi == HO - 1 and kk == 1),
                        )
            filler(3)
            osb = op.tile([OUT, BS], F32, tag="osb")
            nc.scalar.copy(osb[:, :cols], ps3[0:OUT, :cols])
            nc.sync.dma_start(outt[:, off : off + cols], osb[:, :cols])
            filler(3)

        def filler(n):
            # tiny no-dependency matmuls sprinkled through the drain: they
            # execute the moment PE reaches them, keeping the HAM activity
            # window non-idle so the PE clock stays at 2.4 GHz while real
            # work waits on norm chains
            fil = pp.tile([128, BS], F32, tag="mm")
            for i in range(n):
                nc.tensor.matmul(
                    fil[0:32, 0:128],
                    lhsT=ones[:, 0:32],
                    rhs=ones[:],
                    start=True,
                    stop=True,
                    skip_group_check=True,
                )

        nc.vector.memset(ones[:], 1.0 / H)

        # software pipeline over items; superstep s emits
        #   [L1(s)] [norm L2(s-2)] [L2(s-1)] [norm L1(s)] [L3(s-2)]
        # so each ones-matmul sits behind a full matmul block (its DVE
        # reduce-tree inputs are long since ready -> no PE stall), and the
        # L3 of item s-2 runs after norm L2(s-2) completed mid-superstep.
        st1: dict[int, object] = {}
        st2: dict[int, object] = {}
        h1s: dict[int, object] = {}
        h2s: dict[int, object] = {}
        xq: dict[int, tuple] = {}
        for s in range(NI + 2):
            if s == 0:
                xq[0] = load_x(*items[0], interleave_w1=True)
            if s < NI:
                off, cols = items[s]
                st1[s] = l1_mm(*xq.pop(s), cols)
            if s + 1 < NI:
                # prefetch next item's x now, so in the DMA queue it sits
                # before this superstep's other loads (w2/w3 at s==0)
                xq[s + 1] = load_x(*items[s + 1])
            if s == 0:
                load_l23_weights()
            if 2 <= s and (s - 2) not in FINE:
                c = s - 2
                h2s[c] = norm(st2.pop(c), g2sb, hq2, "2", items[c][1])
            if 1 <= s <= NI:
                c = s - 1
                fine2 = c in FINE
                if s >= NI - 1:
                    filler(3)
                t2 = l2_mm_half(h1s[c], items[c][1], 0, fine=fine2)
            if s < NI:
                h1s[s] = norm(st1.pop(s), g1sb, hq1, "1", cols, fine=(s in FINE))
            if 1 <= s <= NI:
                c = s - 1
                t2 = l2_mm_half(h1s.pop(c), items[c][1], 1, fine=fine2, tiles=t2)
                if fine2:
                    st2[c] = (t2[0], t2[4])
                else:
                    # L2 square/reduce tree emitted AFTER norm L1's DVE chain:
                    # keeps rsqrt->prescale->gelu to h1out off the DVE
                    # critical path (the L2 ones-matmul only fires next
                    # superstep)
                    st2[c] = (t2[0], sq_tree(t2[0], items[c][1], "2"))
            if 2 <= s:
                c = s - 2
                if c in FINE:
                    # drain path: fused fine norm2+L3, emitted after the B
                    # block so its strip matmuls never block L2 work; with
                    # HAM fillers so the idle-ish drain stays at 2.4 GHz
                    if s >= NI:
                        filler(3)
                    norm2_l3_fused(st2.pop(c), items[c][0], items[c][1])
                else:
                    l3(h2s.pop(c), items[c][0], items[c][1])

    nc.compile()
    return nc


def _quant(w):
    s = max(float(np.mean(np.abs(w))), EPS_Q)
    return np.clip(np.round(w / s), -1.0, 1.0)


def kernel(x, w1, g1, w2, g2, w3):
    global LAST_RESULTS
    bf = ml_dtypes.bfloat16

    w1q = _quant(np.asarray(w1, np.float32))  # [H, IN]
    w2q = _quant(np.asarray(w2, np.float32))  # [H, H]
    w3q = _quant(np.asarray(w3, np.float32))  # [OUT, H]

    f8 = ml_dtypes.float8_e4m3fn  # ternary weights are exact in e4m3
    w1T = w1q.T.astype(f8)  # [IN, H]
    w1t_np = np.ascontiguousarray(w1T[: KF * 128])
    w1lt_np = np.zeros([128, 2 * 128], dtype=f8)
    for j in range(2):
        for g_ in range(4):
            oi = 4 * j + g_
            w1lt_np[32 * g_ : 32 * g_ + KL, j * 128 : (j + 1) * 128] = w1T[
                KF * 128 :, oi * 128 : (oi + 1) * 128
            ]
    w2t_np = np.ascontiguousarray(w2q.T.astype(f8))
    # w3 packed so each partition's row is contiguous: [128, K2*OUT]
    w3t_np = np.ascontiguousarray(
        w3q.T.astype(f8).reshape(K2, 128, OUT).transpose(1, 0, 2).reshape(128, K2 * OUT)
    )
    g1_np = np.ascontiguousarray(np.asarray(g1, np.float32).reshape(HO, 128).T)
    g2_np = np.ascontiguousarray(np.asarray(g2, np.float32).reshape(HO, 128).T)

    xT = np.asarray(x, np.float32).T.astype(bf)  # [IN, B]
    xt_np = np.ascontiguousarray(xT[: KF * 128])
    xlt_np = np.zeros([128, B], dtype=bf)
    for g_ in range(4):
        xlt_np[32 * g_ : 32 * g_ + KL] = xT[KF * 128 :]

    g_is_one = bool(np.all(np.asarray(g1) == 1.0) and np.all(np.asarray(g2) == 1.0))
    key = ("nc", g_is_one)
    if key not in _cache:
        _cache[key] = _build(g_is_one)
    nc = _cache[key]

    in_maps = []
    for i in range(N_CORES):
        sl = slice(i * BPC, (i + 1) * BPC)
        in_maps.append(
            {
                "xt": np.ascontiguousarray(xt_np[:, sl]),
                "xlt": np.ascontiguousarray(xlt_np[:, sl]),
                "w1t": w1t_np,
                "w1lt": w1lt_np,
                "w2t": w2t_np,
                "w3t": w3t_np,
                "g1": g1_np,
                "g2": g2_np,
            }
        )

    res = run_bass_kernel_spmd(nc, in_maps, core_ids=list(range(N_CORES)))
    LAST_RESULTS = res

    out = np.empty([B, OUT], dtype=np.float32)
    for i in range(N_CORES):
        out[i * BPC : (i + 1) * BPC] = res.results[i]["outt"].T
    return out
